# revision 7
# baseline (speedup 1.0000x reference)
"""Allegro GNN layer on 8 Trainium2 NeuronCores — v2 (pipelined).

Chunk-level software pipeline: while chunk c's scatter/gather/CG products run
(PE light, DVE/GpSimd heavy), the MLP + equivariant output of chunk c-1 keep
the TensorEngine dense so the HAM clock stays warm.  MLP weights are kept
stationary across 4 edge groups to amortize LDWEIGHTS.
"""

import math
import os
import sys

import numpy as np

sys.path.insert(0, "/opt/trn_rl_repo")

NUM_NODES = 8192
MUL = 64
ENV_P = 6
EPS = 1.0 / math.sqrt(17.0)
N_CORES = 8
CHUNK_NODES = 128
CHUNKS_PER_CORE = 8

ENV_A = -(ENV_P + 1) * (ENV_P + 2) / 2.0
ENV_B = float(ENV_P * (ENV_P + 2))
ENV_C = -ENV_P * (ENV_P + 1) / 2.0

_GRAPH_CACHE = {}
_FLAGS_PATCHED = [False]


def _patch_cc_flags():
    if _FLAGS_PATCHED[0]:
        return
    try:
        from concourse.compiler_utils import (get_compiler_flags,
                                              set_compiler_flags)
        flags = get_compiler_flags()
        out = []
        for f in flags:
            if f.startswith("--tensorizer-options=") and "DataLocalityOpt" not in f:
                f = f.rstrip() + " --skip-pass=DataLocalityOpt "
            if (os.environ.get("LDW_OPT", "0") == "1"
                    and f.startswith("--internal-backend-options=")):
                f = f.replace("--enable-ldw-opt=false", "--enable-ldw-opt=true")
            out.append(f)
        set_compiler_flags(out)
    except Exception:
        pass
    _FLAGS_PATCHED[0] = True


def _build_graph(T):
    if T in _GRAPH_CACHE:
        return _GRAPH_CACHE[T]
    _patch_cc_flags()

    import concourse.tile as tile
    from concourse import bacc, mybir
    from contextlib import ExitStack

    f32 = mybir.dt.float32
    bf16 = mybir.dt.bfloat16
    NT = CHUNKS_PER_CORE * T
    E_PAD = NT * 128
    SPC = T * 128
    groups = []
    e0 = 0
    while e0 < SPC:
        eg = min(512, SPC - e0)
        groups.append((e0, eg))
        e0 += eg
    NG = len(groups)
    main_gis = [gi for gi, (_, eg) in enumerate(groups) if eg == 512][:4]
    tail_gis = [gi for gi in range(NG) if gi not in main_gis]

    nc = bacc.Bacc("TRN2", target_bir_lowering=False, debug=False,
                   num_devices=N_CORES)

    xT = nc.dram_tensor("xT", [512, E_PAD], bf16, kind="ExternalInput").ap()
    vec = nc.dram_tensor("vec", [E_PAD, 3], f32, kind="ExternalInput").ap()
    VTm = nc.dram_tensor("VTm", [64, 4, E_PAD], bf16, kind="ExternalInput").ap()
    S = nc.dram_tensor("S", [NT, 128, 128], bf16, kind="ExternalInput").ap()
    STg = nc.dram_tensor("STg", [CHUNKS_PER_CORE, 128, SPC], bf16,
                         kind="ExternalInput").ap()
    W1p = nc.dram_tensor("W1p", [128, 5, 512], bf16, kind="ExternalInput").ap()
    W2p = nc.dram_tensor("W2p", [128, 4, 512], bf16, kind="ExternalInput").ap()
    W3p = nc.dram_tensor("W3p", [128, 4, 512], bf16, kind="ExternalInput").ap()
    Wwp = nc.dram_tensor("Wwp", [128, 4, 64], bf16, kind="ExternalInput").ap()
    WLp = nc.dram_tensor("WLp", [64, 3, 64], bf16, kind="ExternalInput").ap()
    out1 = nc.dram_tensor("out1", [E_PAD, 512], bf16, kind="ExternalOutput").ap()
    out2 = nc.dram_tensor("out2", [64, 3, E_PAD], bf16, kind="ExternalOutput").ap()

    MM = mybir.AluOpType.mult
    AD = mybir.AluOpType.add
    SUB = mybir.AluOpType.subtract
    Silu = mybir.ActivationFunctionType.Silu
    Copy = mybir.ActivationFunctionType.Copy

    with tile.TileContext(nc) as tc:
        with ExitStack() as stack:
            consts = stack.enter_context(tc.tile_pool(name="consts", bufs=1))
            w1s = consts.tile([128, 5, 512], bf16)
            nc.sync.dma_start(w1s[:], W1p[:])
            w2s = consts.tile([128, 4, 512], bf16)
            nc.sync.dma_start(w2s[:], W2p[:])
            w3s = consts.tile([128, 4, 512], bf16)
            nc.sync.dma_start(w3s[:], W3p[:])
            wls = consts.tile([64, 3, 64], bf16)
            nc.sync.dma_start(wls[:], WLp[:])

            res = stack.enter_context(tc.tile_pool(name="res", bufs=1))
            w_all = res.tile([128, NT * 64], bf16)
            y_all = res.tile([128, NT, 4], bf16)
            env_all = res.tile([128, NT], f32)

            # ---- Phase A ------------------------------------------------
            with ExitStack() as st_a:
                pA = st_a.enter_context(tc.tile_pool(name="pA", bufs=3))
                pAc = st_a.enter_context(tc.tile_pool(name="pAc", bufs=1))
                pAw = st_a.enter_context(
                    tc.tile_pool(name="pAw", bufs=4, space="PSUM"))
                wws = pAc.tile([128, 4, 64], bf16)
                nc.sync.dma_start(wws[:], Wwp[:])
                for s in range(NT // 8):
                    xa = pA.tile([128, 4, 8, 128], bf16)
                    for kb in range(4):
                        nc.sync.dma_start(
                            xa[:, kb],
                            xT[kb * 128:(kb + 1) * 128,
                               s * 1024:(s + 1) * 1024].rearrange(
                                   "p (g e) -> p g e", g=8))
                    vc = pA.tile([128, 8, 3], f32)
                    nc.sync.dma_start(
                        vc[:],
                        vec[s * 1024:(s + 1) * 1024, :].rearrange(
                            "(g p) d -> p g d", p=128))
                    for es in range(8):
                        pw = pAw.tile([128, 64], f32)
                        for kb in range(4):
                            nc.tensor.matmul(
                                pw[:], xa[:, kb, es], wws[:, kb],
                                start=(kb == 0), stop=(kb == 3))
                        nc.vector.tensor_copy(
                            w_all[:, (8 * s + es) * 64:(8 * s + es + 1) * 64],
                            pw[:])
                    sq = pA.tile([128, 8, 3], f32)
                    nc.vector.tensor_tensor(sq[:], vc[:], vc[:], op=MM)
                    r2 = pA.tile([128, 8], f32)
                    nc.vector.tensor_reduce(
                        r2[:], sq[:], axis=mybir.AxisListType.X, op=AD)
                    r = pA.tile([128, 8], f32)
                    nc.scalar.sqrt(r[:], r2[:])
                    rinv = pA.tile([128, 8], f32)
                    nc.vector.reciprocal(rinv[:], r[:])
                    rinv3 = pA.tile([128, 8], f32)
                    nc.scalar.mul(rinv3[:], rinv[:], math.sqrt(3.0))
                    nc.vector.memset(y_all[:, 8 * s:8 * s + 8, 0], 1.0)
                    nc.vector.tensor_tensor(
                        y_all[:, 8 * s:8 * s + 8, 1:4], vc[:],
                        rinv3[:].rearrange("p (g o) -> p g o", o=1)
                        .to_broadcast([128, 8, 3]), op=MM)
                    inner = pA.tile([128, 8], f32)
                    nc.vector.tensor_scalar(
                        inner[:], r[:], scalar1=ENV_B, scalar2=ENV_A,
                        op0=MM, op1=AD)
                    nc.vector.scalar_tensor_tensor(
                        inner[:], in0=r2[:], scalar=ENV_C, in1=inner[:],
                        op0=MM, op1=AD)
                    u3 = pA.tile([128, 8], f32)
                    nc.vector.tensor_tensor(u3[:], r2[:], r[:], op=MM)
                    u6 = pA.tile([128, 8], f32)
                    nc.vector.tensor_tensor(u6[:], u3[:], u3[:], op=MM)
                    poly = pA.tile([128, 8], f32)
                    nc.vector.tensor_tensor(poly[:], u6[:], inner[:], op=MM)
                    nc.vector.tensor_scalar(
                        poly[:], poly[:], scalar1=1.0, scalar2=None, op0=AD)
                    mask = pA.tile([128, 8], f32)
                    nc.vector.tensor_scalar(
                        mask[:], r2[:], scalar1=1.0,
                        scalar2=1.0 / math.sqrt(512.0),
                        op0=mybir.AluOpType.is_lt, op1=MM)
                    nc.vector.tensor_tensor(
                        env_all[:, 8 * s:8 * s + 8], poly[:], mask[:], op=MM)

            # ---- Phases B/C/D, chunk-pipelined --------------------------
            with ExitStack() as st_b:
                pStg = st_b.enter_context(tc.tile_pool(name="pStg", bufs=2))
                pS = st_b.enter_context(tc.tile_pool(name="pS", bufs=3))
                pWy = st_b.enter_context(tc.tile_pool(name="pWy", bufs=3))
                pAgg = st_b.enter_context(tc.tile_pool(name="pAgg", bufs=2))
                pAggP = st_b.enter_context(
                    tc.tile_pool(name="pAggP", bufs=1, space="PSUM"))
                pGP = st_b.enter_context(
                    tc.tile_pool(name="pGP", bufs=3, space="PSUM"))
                pG = st_b.enter_context(tc.tile_pool(name="pG", bufs=3))
                pV = st_b.enter_context(tc.tile_pool(name="pV", bufs=2))
                pX = st_b.enter_context(tc.tile_pool(name="pX", bufs=NG))
                pCG = st_b.enter_context(tc.tile_pool(name="pCG", bufs=2))
                pSc = st_b.enter_context(
                    tc.tile_pool(name="pSc", bufs=2 * NG + 1))
                pVo = st_b.enter_context(tc.tile_pool(name="pVo", bufs=1))
                pH1 = st_b.enter_context(tc.tile_pool(name="pH1", bufs=NG))
                pH2 = st_b.enter_context(tc.tile_pool(name="pH2", bufs=NG))
                pMP = st_b.enter_context(
                    tc.tile_pool(name="pMP", bufs=4, space="PSUM"))
                pXo = st_b.enter_context(tc.tile_pool(name="pXo", bufs=2))

                def emit_front1(c):
                    """scatter for chunk c."""
                    stg = pStg.tile([128, SPC], bf16)
                    nc.sync.dma_start(stg[:], STg[c])
                    pagg = pAggP.tile([128, 256], f32)
                    for lt in range(T):
                        t = c * T + lt
                        st_ = pS.tile([128, 128], bf16)
                        nc.sync.dma_start(st_[:], S[t])
                        wy = pWy.tile([128, 4, 64], bf16)
                        nc.vector.tensor_tensor(
                            wy[:],
                            y_all[:, t, :].rearrange("p (k o) -> p k o", o=1)
                            .to_broadcast([128, 4, 64]),
                            w_all[:, t * 64:(t + 1) * 64]
                            .rearrange("p (o m) -> p o m", o=1)
                            .to_broadcast([128, 4, 64]),
                            op=MM)
                        nc.tensor.matmul(
                            pagg[:], st_[:],
                            wy[:].rearrange("p a b -> p (a b)"),
                            start=(lt == 0), stop=(lt == T - 1))
                    aggc = pAgg.tile([128, 256], bf16)
                    nc.vector.tensor_copy(aggc[:], pagg[:])
                    return {"c": c, "stg": stg, "aggc": aggc,
                            "prods": [], "sc4": []}

                def emit_front2(state):
                    """gather + CG products for chunk c (PE-light)."""
                    c = state["c"]
                    stg = state["stg"]
                    aggc = state["aggc"]
                    for gi, (e0_, eg) in enumerate(groups):
                        gsl = slice(c * SPC + e0_, c * SPC + e0_ + eg)
                        wyg = pG.tile([64, 4, 512], bf16)
                        for k in range(4):
                            pg = pGP.tile([64, 512], f32, tag="gv")
                            nc.tensor.matmul(
                                pg[:, :eg], aggc[:, k * 64:(k + 1) * 64],
                                stg[:, e0_:e0_ + eg], start=True, stop=True)
                            nc.vector.tensor_copy(wyg[:, k, :eg], pg[:, :eg])
                        vt = pV.tile([64, 4, 512], bf16)
                        nc.sync.dma_start(vt[:, :, :eg], VTm[:, :, gsl])
                        a0 = wyg[:, 0, :eg]
                        ax, ay, az = (wyg[:, 1, :eg], wyg[:, 2, :eg],
                                      wyg[:, 3, :eg])
                        b0 = vt[:, 0, :eg]
                        bx, by, bz = (vt[:, 1, :eg], vt[:, 2, :eg],
                                      vt[:, 3, :eg])
                        sc4 = pSc.tile([128, 512], bf16)
                        nc.vector.tensor_tensor(sc4[0:64, :eg], a0, b0, op=MM)
                        s2t = pCG.tile([64, 512], bf16, tag="s2")
                        tm = pCG.tile([64, 512], bf16, tag="tm")
                        nc.vector.tensor_tensor(s2t[:, :eg], ax, bx, op=MM)
                        nc.vector.tensor_tensor(tm[:, :eg], ay, by, op=MM)
                        nc.vector.tensor_tensor(
                            s2t[:, :eg], s2t[:, :eg], tm[:, :eg], op=AD)
                        nc.vector.tensor_tensor(tm[:, :eg], az, bz, op=MM)
                        nc.vector.tensor_tensor(
                            sc4[64:128, :eg], s2t[:, :eg], tm[:, :eg], op=AD)
                        comps = [(bx, ax, (ay, bz, az, by)),
                                 (by, ay, (az, bx, ax, bz)),
                                 (bz, az, (ax, by, ay, bx))]
                        prods = []
                        for ci, (b1c, a1c, (cp, cq, cr_, cs_)) in enumerate(comps):
                            pr = pCG.tile([64, 3, 512], bf16, tag=f"pr{ci}", bufs=6)
                            nc.gpsimd.tensor_tensor(
                                pr[:, 0, :eg], a0, b1c, op=MM)
                            nc.gpsimd.tensor_tensor(
                                pr[:, 1, :eg], a1c, b0, op=MM)
                            gtmp = pCG.tile([64, 512], bf16, tag="gtmp")
                            nc.gpsimd.tensor_tensor(
                                pr[:, 2, :eg], cp, cq, op=MM)
                            nc.gpsimd.tensor_tensor(
                                gtmp[:, :eg], cr_, cs_, op=MM)
                            nc.gpsimd.tensor_tensor(
                                pr[:, 2, :eg], pr[:, 2, :eg], gtmp[:, :eg],
                                op=SUB)
                            prods.append(pr)
                        state["prods"].append(prods)
                        state["sc4"].append(sc4)

                def emit_vout(st):
                    """V_out for chunk st['c']."""
                    c = st["c"]
                    for gi, (e0_, eg) in enumerate(groups):
                        gsl = slice(c * SPC + e0_, c * SPC + e0_ + eg)
                        prods = st["prods"][gi]
                        pvs = [pGP.tile([64, 512], f32, tag="gv",
                                        name=f"pv{i}")
                               for i in range(3)]
                        for b in range(3):
                            for ci in range(3):
                                nc.tensor.matmul(
                                    pvs[ci][:, :eg], wls[:, b],
                                    prods[ci][:, b, :eg],
                                    start=(b == 0), stop=(b == 2))
                        vout = pVo.tile([64, 3, 512], bf16)
                        for ci in range(3):
                            nc.vector.tensor_copy(
                                vout[:, ci, :eg], pvs[ci][:, :eg])
                        nc.scalar.dma_start(out2[:, :, gsl], vout[:, :, :eg])

                def emit_back(st):
                    """MLP for chunk st['c'] (PE-dense)."""
                    c = st["c"]
                    xtds = []
                    for gi, (e0_, eg) in enumerate(groups):
                        gsl = slice(c * SPC + e0_, c * SPC + e0_ + eg)
                        xtd = pX.tile([128, 4, 512], bf16)
                        for kb in range(4):
                            nc.sync.dma_start(
                                xtd[:, kb, :eg],
                                xT[kb * 128:(kb + 1) * 128, gsl])
                        xtds.append(xtd)
                    # W1: weight-stationary across 4 main groups
                    h1s = [pH1.tile([128, 4, 512], bf16, tag="h1", name=f"h1_{i}")
                           for i in range(NG)]
                    for hb in range(4):
                        hsl = slice(hb * 128, (hb + 1) * 128)
                        p1s = {gi: pMP.tile([128, 512], f32, tag="mlp",
                                            name=f"p1_{gi}")
                               for gi in main_gis}
                        for kb in range(5):
                            for gi in main_gis:
                                rhs = (xtds[gi][:, kb, :] if kb < 4
                                       else st["sc4"][gi][:, :])
                                nc.tensor.matmul(
                                    p1s[gi][:], w1s[:, kb, hsl], rhs,
                                    start=(kb == 0), stop=(kb == 4))
                        for gi in main_gis:
                            nc.scalar.activation(
                                h1s[gi][:, hb, :], p1s[gi][:], Silu,
                                scale=1.0 / math.sqrt(640.0))
                    for gi in tail_gis:
                        eg = groups[gi][1]
                        for hb in range(4):
                            hsl = slice(hb * 128, (hb + 1) * 128)
                            p1 = pMP.tile([128, 512], f32, tag="mlp")
                            for kb in range(5):
                                rhs = (xtds[gi][:, kb, :eg] if kb < 4
                                       else st["sc4"][gi][:, :eg])
                                nc.tensor.matmul(
                                    p1[:, :eg], w1s[:, kb, hsl], rhs,
                                    start=(kb == 0), stop=(kb == 4))
                            nc.scalar.activation(
                                h1s[gi][:, hb, :eg], p1[:, :eg], Silu,
                                scale=1.0 / math.sqrt(640.0))
                    # W2
                    h2s = [pH2.tile([128, 4, 512], bf16, tag="h2", name=f"h2_{i}")
                           for i in range(NG)]
                    for hb in range(4):
                        hsl = slice(hb * 128, (hb + 1) * 128)
                        p2s = {gi: pMP.tile([128, 512], f32, tag="mlp",
                                            name=f"p2_{gi}")
                               for gi in main_gis}
                        for kb in range(4):
                            for gi in main_gis:
                                nc.tensor.matmul(
                                    p2s[gi][:], w2s[:, kb, hsl],
                                    h1s[gi][:, kb, :],
                                    start=(kb == 0), stop=(kb == 3))
                        for gi in main_gis:
                            nc.scalar.activation(
                                h2s[gi][:, hb, :], p2s[gi][:], Silu,
                                scale=1.0 / math.sqrt(512.0))
                    for gi in tail_gis:
                        eg = groups[gi][1]
                        for hb in range(4):
                            hsl = slice(hb * 128, (hb + 1) * 128)
                            p2 = pMP.tile([128, 512], f32, tag="mlp")
                            for kb in range(4):
                                nc.tensor.matmul(
                                    p2[:, :eg], w2s[:, kb, hsl],
                                    h1s[gi][:, kb, :eg],
                                    start=(kb == 0), stop=(kb == 3))
                            nc.scalar.activation(
                                h2s[gi][:, hb, :eg], p2[:, :eg], Silu,
                                scale=1.0 / math.sqrt(512.0))
                    # W3 + envelope
                    for gi, (e0_, eg) in enumerate(groups):
                        for es in range(eg // 128):
                            gt = (c * SPC + e0_) // 128 + es
                            p3 = pMP.tile([128, 512], f32, tag="mlp")
                            for kb in range(4):
                                nc.tensor.matmul(
                                    p3[:],
                                    h2s[gi][:, kb, es * 128:(es + 1) * 128],
                                    w3s[:, kb],
                                    start=(kb == 0), stop=(kb == 3))
                            xo = pXo.tile([128, 512], bf16)
                            nc.scalar.activation(
                                xo[:], p3[:], Copy,
                                scale=env_all[:, gt:gt + 1])
                            nc.scalar.dma_start(
                                out1[gt * 128:(gt + 1) * 128, :], xo[:])

                prev = None
                for c in range(CHUNKS_PER_CORE):
                    st = emit_front1(c)
                    if prev is not None:
                        emit_vout(prev)
                    emit_front2(st)
                    if prev is not None:
                        emit_back(prev)
                    prev = st
                emit_vout(prev)
                emit_back(prev)
    nc.compile()
    _GRAPH_CACHE[T] = (nc, NT, E_PAD, SPC)
    return _GRAPH_CACHE[T]


def _prep_host(vectors, x, V, senders, W_w, W1, W2, W3, W_lin):
    import ml_dtypes
    bf = ml_dtypes.bfloat16
    E = senders.shape[0]
    gchunk = senders.astype(np.int64) // CHUNK_NODES
    order = np.argsort(gchunk, kind="stable")
    counts = np.bincount(gchunk, minlength=64)
    T = max(18, int(math.ceil(counts.max() / 128.0)))
    SPC = T * 128
    NT = CHUNKS_PER_CORE * T
    E_PAD = NT * 128

    sg = gchunk[order]
    starts = np.zeros(64, np.int64)
    starts[1:] = np.cumsum(counts)[:-1]
    within = np.arange(E, dtype=np.int64) - starts[sg]
    core = sg // CHUNKS_PER_CORE
    lchunk = sg % CHUNKS_PER_CORE
    slot = lchunk * SPC + within

    W1s = W1.astype(np.float64)
    W1s[576:640] /= math.sqrt(3.0)
    W1p = np.ascontiguousarray(
        W1s.reshape(5, 128, 512).transpose(1, 0, 2)).astype(bf)
    W2p = np.ascontiguousarray(
        W2.reshape(4, 128, 512).transpose(1, 0, 2)).astype(bf)
    W3p = np.ascontiguousarray(
        W3.reshape(4, 128, 512).transpose(1, 0, 2)).astype(bf)
    Wwp = np.ascontiguousarray(
        (W_w.astype(np.float64) * (EPS / math.sqrt(512.0)))
        .reshape(4, 128, 64).transpose(1, 0, 2)).astype(bf)
    WLs = W_lin.astype(np.float64) / math.sqrt(192.0)
    WLs[128:192] /= math.sqrt(2.0)
    WLp = np.ascontiguousarray(
        WLs.reshape(3, 64, 64).transpose(1, 0, 2)).astype(bf)

    in_maps = []
    metas = []
    for d in range(N_CORES):
        m = core == d
        eidx = order[m]
        sl = slot[m]
        fill = eidx[0] if len(eidx) else 0
        xs = np.empty((E_PAD, 512), np.float32)
        xs[:] = x[fill]
        xs[sl] = x[eidx]
        vs = np.empty((E_PAD, 3), np.float32)
        vs[:] = vectors[fill]
        vs[sl] = vectors[eidx]
        Vs = np.empty((E_PAD, 64, 4), np.float32)
        Vs[:] = V[fill]
        Vs[sl] = V[eidx]
        xTc = np.ascontiguousarray(xs.T).astype(bf)
        VTmc = np.ascontiguousarray(Vs.transpose(1, 2, 0)).astype(bf)
        Sc = np.zeros((NT, 128, 128), bf)
        col = (senders[eidx] % CHUNK_NODES).astype(np.int64)
        Sc[sl // 128, sl % 128, col] = 1.0
        STgc = np.zeros((CHUNKS_PER_CORE, 128, SPC), bf)
        STgc[sl // SPC, col, sl % SPC] = 1.0
        in_maps.append({
            "xT": xTc, "vec": vs, "VTm": VTmc, "S": Sc, "STg": STgc,
            "W1p": W1p, "W2p": W2p, "W3p": W3p, "Wwp": Wwp, "WLp": WLp,
        })
        metas.append((eidx, sl))
    return T, in_maps, metas


def kernel(vectors, x, V, senders, W_w, W1, W2, W3, W_lin):
    vectors = np.asarray(vectors, np.float32)
    x = np.asarray(x, np.float32)
    V = np.asarray(V, np.float32)
    senders = np.asarray(senders)
    W_w = np.asarray(W_w, np.float32)
    W1 = np.asarray(W1, np.float32)
    W2 = np.asarray(W2, np.float32)
    W3 = np.asarray(W3, np.float32)
    W_lin = np.asarray(W_lin, np.float32)

    T, in_maps, metas = _prep_host(
        vectors, x, V, senders, W_w, W1, W2, W3, W_lin)
    nc, NT, E_PAD, SPC = _build_graph(T)

    from concourse.bass_utils import run_bass_kernel_spmd
    res = run_bass_kernel_spmd(nc, in_maps, core_ids=list(range(N_CORES)))

    E = senders.shape[0]
    x_out = np.empty((E, 512), np.float32)
    V_out = np.empty((E, 64, 3), np.float32)
    for d in range(N_CORES):
        eidx, sl = metas[d]
        o1 = res.results[d]["out1"]
        o2 = res.results[d]["out2"]
        x_out[eidx] = o1[sl].astype(np.float32)
        V_out[eidx] = o2.transpose(2, 0, 1)[sl].astype(np.float32)
    return x_out, V_out


# revision 8
# speedup vs baseline: 1.2404x; 1.2404x over previous
"""Allegro GNN layer on 8 Trainium2 NeuronCores — v2 (pipelined).

Chunk-level software pipeline: while chunk c's scatter/gather/CG products run
(PE light, DVE/GpSimd heavy), the MLP + equivariant output of chunk c-1 keep
the TensorEngine dense so the HAM clock stays warm.  MLP weights are kept
stationary across 4 edge groups to amortize LDWEIGHTS.
"""

import math
import os
import sys

import numpy as np

sys.path.insert(0, "/opt/trn_rl_repo")

NUM_NODES = 8192
MUL = 64
ENV_P = 6
EPS = 1.0 / math.sqrt(17.0)
N_CORES = 8
CHUNK_NODES = 128
CHUNKS_PER_CORE = 8

ENV_A = -(ENV_P + 1) * (ENV_P + 2) / 2.0
ENV_B = float(ENV_P * (ENV_P + 2))
ENV_C = -ENV_P * (ENV_P + 1) / 2.0

_GRAPH_CACHE = {}
_FLAGS_PATCHED = [False]


def _patch_cc_flags():
    if _FLAGS_PATCHED[0]:
        return
    try:
        from concourse.compiler_utils import (get_compiler_flags,
                                              set_compiler_flags)
        flags = get_compiler_flags()
        out = []
        for f in flags:
            if f.startswith("--tensorizer-options=") and "DataLocalityOpt" not in f:
                f = f.rstrip() + " --skip-pass=DataLocalityOpt "
            if (os.environ.get("LDW_OPT", "0") == "1"
                    and f.startswith("--internal-backend-options=")):
                f = f.replace("--enable-ldw-opt=false", "--enable-ldw-opt=true")
            out.append(f)
        set_compiler_flags(out)
    except Exception:
        pass
    _FLAGS_PATCHED[0] = True


def _build_graph(T):
    if T in _GRAPH_CACHE:
        return _GRAPH_CACHE[T]
    _patch_cc_flags()

    import concourse.tile as tile
    from concourse import bacc, mybir
    from contextlib import ExitStack

    f32 = mybir.dt.float32
    bf16 = mybir.dt.bfloat16
    NT = CHUNKS_PER_CORE * T
    E_PAD = NT * 128
    SPC = T * 128
    groups = []
    e0 = 0
    while e0 < SPC:
        eg = min(512, SPC - e0)
        groups.append((e0, eg))
        e0 += eg
    NG = len(groups)
    main_gis = [gi for gi, (_, eg) in enumerate(groups) if eg == 512][:4]
    tail_gis = [gi for gi in range(NG) if gi not in main_gis]

    nc = bacc.Bacc("TRN2", target_bir_lowering=False, debug=False,
                   num_devices=N_CORES)

    xT = nc.dram_tensor("xT", [512, E_PAD], bf16, kind="ExternalInput").ap()
    vec = nc.dram_tensor("vec", [E_PAD, 3], f32, kind="ExternalInput").ap()
    VTm = nc.dram_tensor("VTm", [64, 4, E_PAD], bf16, kind="ExternalInput").ap()
    S = nc.dram_tensor("S", [NT, 128, 128], bf16, kind="ExternalInput").ap()
    STg = nc.dram_tensor("STg", [CHUNKS_PER_CORE, 128, SPC], bf16,
                         kind="ExternalInput").ap()
    W1p = nc.dram_tensor("W1p", [128, 5, 512], bf16, kind="ExternalInput").ap()
    W2p = nc.dram_tensor("W2p", [128, 4, 512], bf16, kind="ExternalInput").ap()
    W3p = nc.dram_tensor("W3p", [128, 4, 512], bf16, kind="ExternalInput").ap()
    Wwp = nc.dram_tensor("Wwp", [128, 4, 64], bf16, kind="ExternalInput").ap()
    WLp = nc.dram_tensor("WLp", [64, 3, 64], bf16, kind="ExternalInput").ap()
    out1 = nc.dram_tensor("out1", [E_PAD, 512], bf16, kind="ExternalOutput").ap()
    out2 = nc.dram_tensor("out2", [64, 3, E_PAD], bf16, kind="ExternalOutput").ap()

    MM = mybir.AluOpType.mult
    AD = mybir.AluOpType.add
    SUB = mybir.AluOpType.subtract
    Silu = mybir.ActivationFunctionType.Silu
    Copy = mybir.ActivationFunctionType.Copy

    with tile.TileContext(nc) as tc:
        with ExitStack() as stack:
            consts = stack.enter_context(tc.tile_pool(name="consts", bufs=1))
            w1s = consts.tile([128, 5, 512], bf16)
            nc.sync.dma_start(w1s[:], W1p[:])
            w2s = consts.tile([128, 4, 512], bf16)
            nc.sync.dma_start(w2s[:], W2p[:])
            w3s = consts.tile([128, 4, 512], bf16)
            nc.sync.dma_start(w3s[:], W3p[:])
            wls = consts.tile([64, 3, 64], bf16)
            nc.sync.dma_start(wls[:], WLp[:])

            res = stack.enter_context(tc.tile_pool(name="res", bufs=1))
            w_all = res.tile([128, NT * 64], bf16)
            y_all = res.tile([128, NT, 4], bf16)
            env_all = res.tile([128, NT], f32)

            # ---- Phase A ------------------------------------------------
            with ExitStack() as st_a:
                pA = st_a.enter_context(tc.tile_pool(name="pA", bufs=3))
                pAc = st_a.enter_context(tc.tile_pool(name="pAc", bufs=1))
                pAw = st_a.enter_context(
                    tc.tile_pool(name="pAw", bufs=4, space="PSUM"))
                wws = pAc.tile([128, 4, 64], bf16)
                nc.sync.dma_start(wws[:], Wwp[:])
                for s in range(NT // 8):
                    xa = pA.tile([128, 4, 8, 128], bf16)
                    for kb in range(4):
                        nc.sync.dma_start(
                            xa[:, kb],
                            xT[kb * 128:(kb + 1) * 128,
                               s * 1024:(s + 1) * 1024].rearrange(
                                   "p (g e) -> p g e", g=8))
                    vc = pA.tile([128, 8, 3], f32)
                    nc.sync.dma_start(
                        vc[:],
                        vec[s * 1024:(s + 1) * 1024, :].rearrange(
                            "(g p) d -> p g d", p=128))
                    for es in range(8):
                        pw = pAw.tile([128, 64], f32)
                        for kb in range(4):
                            nc.tensor.matmul(
                                pw[:], xa[:, kb, es], wws[:, kb],
                                start=(kb == 0), stop=(kb == 3))
                        nc.vector.tensor_copy(
                            w_all[:, (8 * s + es) * 64:(8 * s + es + 1) * 64],
                            pw[:])
                    sq = pA.tile([128, 8, 3], f32)
                    nc.vector.tensor_tensor(sq[:], vc[:], vc[:], op=MM)
                    r2 = pA.tile([128, 8], f32)
                    nc.vector.tensor_reduce(
                        r2[:], sq[:], axis=mybir.AxisListType.X, op=AD)
                    r = pA.tile([128, 8], f32)
                    nc.scalar.sqrt(r[:], r2[:])
                    rinv = pA.tile([128, 8], f32)
                    nc.vector.reciprocal(rinv[:], r[:])
                    rinv3 = pA.tile([128, 8], f32)
                    nc.scalar.mul(rinv3[:], rinv[:], math.sqrt(3.0))
                    nc.vector.memset(y_all[:, 8 * s:8 * s + 8, 0], 1.0)
                    nc.vector.tensor_tensor(
                        y_all[:, 8 * s:8 * s + 8, 1:4], vc[:],
                        rinv3[:].rearrange("p (g o) -> p g o", o=1)
                        .to_broadcast([128, 8, 3]), op=MM)
                    inner = pA.tile([128, 8], f32)
                    nc.vector.tensor_scalar(
                        inner[:], r[:], scalar1=ENV_B, scalar2=ENV_A,
                        op0=MM, op1=AD)
                    nc.vector.scalar_tensor_tensor(
                        inner[:], in0=r2[:], scalar=ENV_C, in1=inner[:],
                        op0=MM, op1=AD)
                    u3 = pA.tile([128, 8], f32)
                    nc.vector.tensor_tensor(u3[:], r2[:], r[:], op=MM)
                    u6 = pA.tile([128, 8], f32)
                    nc.vector.tensor_tensor(u6[:], u3[:], u3[:], op=MM)
                    poly = pA.tile([128, 8], f32)
                    nc.vector.tensor_tensor(poly[:], u6[:], inner[:], op=MM)
                    nc.vector.tensor_scalar(
                        poly[:], poly[:], scalar1=1.0, scalar2=None, op0=AD)
                    mask = pA.tile([128, 8], f32)
                    nc.vector.tensor_scalar(
                        mask[:], r2[:], scalar1=1.0,
                        scalar2=1.0 / math.sqrt(512.0),
                        op0=mybir.AluOpType.is_lt, op1=MM)
                    nc.vector.tensor_tensor(
                        env_all[:, 8 * s:8 * s + 8], poly[:], mask[:], op=MM)

            # ---- Phases B/C/D, chunk-pipelined --------------------------
            with ExitStack() as st_b:
                pStg = st_b.enter_context(tc.tile_pool(name="pStg", bufs=2))
                pS = st_b.enter_context(tc.tile_pool(name="pS", bufs=3))
                pWy = st_b.enter_context(tc.tile_pool(name="pWy", bufs=3))
                pAgg = st_b.enter_context(tc.tile_pool(name="pAgg", bufs=2))
                pAggP = st_b.enter_context(
                    tc.tile_pool(name="pAggP", bufs=1, space="PSUM"))
                pGP = st_b.enter_context(
                    tc.tile_pool(name="pGP", bufs=3, space="PSUM"))
                pG = st_b.enter_context(tc.tile_pool(name="pG", bufs=3))
                pV = st_b.enter_context(tc.tile_pool(name="pV", bufs=2))
                pX = st_b.enter_context(tc.tile_pool(name="pX", bufs=NG))
                pCG = st_b.enter_context(tc.tile_pool(name="pCG", bufs=2))
                pSc = st_b.enter_context(
                    tc.tile_pool(name="pSc", bufs=2 * NG + 1))
                pVo = st_b.enter_context(tc.tile_pool(name="pVo", bufs=1))
                pH1 = st_b.enter_context(tc.tile_pool(name="pH1", bufs=NG))
                pH2 = st_b.enter_context(tc.tile_pool(name="pH2", bufs=NG))
                pMP = st_b.enter_context(
                    tc.tile_pool(name="pMP", bufs=4, space="PSUM"))
                pXo = st_b.enter_context(tc.tile_pool(name="pXo", bufs=2))

                def emit_front1(c):
                    """scatter for chunk c."""
                    stg = pStg.tile([128, SPC], bf16)
                    nc.sync.dma_start(stg[:], STg[c])
                    pagg = pAggP.tile([128, 256], f32)
                    for lt in range(T):
                        t = c * T + lt
                        st_ = pS.tile([128, 128], bf16)
                        nc.sync.dma_start(st_[:], S[t])
                        wy = pWy.tile([128, 4, 64], bf16)
                        nc.vector.tensor_tensor(
                            wy[:],
                            y_all[:, t, :].rearrange("p (k o) -> p k o", o=1)
                            .to_broadcast([128, 4, 64]),
                            w_all[:, t * 64:(t + 1) * 64]
                            .rearrange("p (o m) -> p o m", o=1)
                            .to_broadcast([128, 4, 64]),
                            op=MM)
                        nc.tensor.matmul(
                            pagg[:], st_[:],
                            wy[:].rearrange("p a b -> p (a b)"),
                            start=(lt == 0), stop=(lt == T - 1))
                    aggc = pAgg.tile([128, 256], bf16)
                    nc.vector.tensor_copy(aggc[:], pagg[:])
                    return {"c": c, "stg": stg, "aggc": aggc,
                            "prods": [], "sc4": []}

                def emit_front2(state):
                    """gather + CG products for chunk c (PE-light)."""
                    c = state["c"]
                    stg = state["stg"]
                    aggc = state["aggc"]
                    for gi, (e0_, eg) in enumerate(groups):
                        gsl = slice(c * SPC + e0_, c * SPC + e0_ + eg)
                        wyg = pG.tile([64, 4, 512], bf16)
                        for k in range(4):
                            pg = pGP.tile([64, 512], f32, tag="gv")
                            nc.tensor.matmul(
                                pg[:, :eg], aggc[:, k * 64:(k + 1) * 64],
                                stg[:, e0_:e0_ + eg], start=True, stop=True)
                            nc.vector.tensor_copy(wyg[:, k, :eg], pg[:, :eg])
                        vt = pV.tile([64, 4, 512], bf16)
                        nc.sync.dma_start(vt[:, :, :eg], VTm[:, :, gsl])
                        a0 = wyg[:, 0, :eg]
                        ax, ay, az = (wyg[:, 1, :eg], wyg[:, 2, :eg],
                                      wyg[:, 3, :eg])
                        b0 = vt[:, 0, :eg]
                        bx, by, bz = (vt[:, 1, :eg], vt[:, 2, :eg],
                                      vt[:, 3, :eg])
                        sc4 = pSc.tile([128, 512], bf16)
                        nc.vector.tensor_tensor(sc4[0:64, :eg], a0, b0, op=MM)
                        s2t = pCG.tile([64, 512], bf16, tag="s2")
                        tm = pCG.tile([64, 512], bf16, tag="tm")
                        nc.vector.tensor_tensor(s2t[:, :eg], ax, bx, op=MM)
                        nc.vector.tensor_tensor(tm[:, :eg], ay, by, op=MM)
                        nc.vector.tensor_tensor(
                            s2t[:, :eg], s2t[:, :eg], tm[:, :eg], op=AD)
                        nc.vector.tensor_tensor(tm[:, :eg], az, bz, op=MM)
                        nc.vector.tensor_tensor(
                            sc4[64:128, :eg], s2t[:, :eg], tm[:, :eg], op=AD)
                        comps = [(bx, ax, (ay, bz, az, by)),
                                 (by, ay, (az, bx, ax, bz)),
                                 (bz, az, (ax, by, ay, bx))]
                        prods = []
                        for ci, (b1c, a1c, (cp, cq, cr_, cs_)) in enumerate(comps):
                            pr = pCG.tile([64, 3, 512], bf16, tag=f"pr{ci}", bufs=6)
                            nc.gpsimd.tensor_tensor(
                                pr[:, 0, :eg], a0, b1c, op=MM)
                            nc.gpsimd.tensor_tensor(
                                pr[:, 1, :eg], a1c, b0, op=MM)
                            gtmp = pCG.tile([64, 512], bf16, tag="gtmp")
                            nc.gpsimd.tensor_tensor(
                                pr[:, 2, :eg], cp, cq, op=MM)
                            nc.gpsimd.tensor_tensor(
                                gtmp[:, :eg], cr_, cs_, op=MM)
                            nc.gpsimd.tensor_tensor(
                                pr[:, 2, :eg], pr[:, 2, :eg], gtmp[:, :eg],
                                op=SUB)
                            prods.append(pr)
                        state["prods"].append(prods)
                        state["sc4"].append(sc4)

                def emit_vout(st):
                    """V_out for chunk st['c']."""
                    c = st["c"]
                    for gi, (e0_, eg) in enumerate(groups):
                        gsl = slice(c * SPC + e0_, c * SPC + e0_ + eg)
                        prods = st["prods"][gi]
                        pvs = [pGP.tile([64, 512], f32, tag="gv",
                                        name=f"pv{i}")
                               for i in range(3)]
                        for b in range(3):
                            for ci in range(3):
                                nc.tensor.matmul(
                                    pvs[ci][:, :eg], wls[:, b],
                                    prods[ci][:, b, :eg],
                                    start=(b == 0), stop=(b == 2))
                        vout = pVo.tile([64, 3, 512], bf16)
                        for ci in range(3):
                            nc.vector.tensor_copy(
                                vout[:, ci, :eg], pvs[ci][:, :eg])
                        nc.scalar.dma_start(out2[:, :, gsl], vout[:, :, :eg])

                def emit_back(st):
                    """MLP for chunk st['c'] (PE-dense)."""
                    c = st["c"]
                    xtds = []
                    for gi, (e0_, eg) in enumerate(groups):
                        gsl = slice(c * SPC + e0_, c * SPC + e0_ + eg)
                        xtd = pX.tile([128, 4, 512], bf16)
                        for kb in range(4):
                            nc.sync.dma_start(
                                xtd[:, kb, :eg],
                                xT[kb * 128:(kb + 1) * 128, gsl])
                        xtds.append(xtd)
                    # W1: weight-stationary across 4 main groups
                    h1s = [pH1.tile([128, 4, 512], bf16, tag="h1", name=f"h1_{i}")
                           for i in range(NG)]
                    for hb in range(4):
                        hsl = slice(hb * 128, (hb + 1) * 128)
                        p1s = {gi: pMP.tile([128, 512], f32, tag="mlp",
                                            name=f"p1_{gi}")
                               for gi in main_gis}
                        for kb in range(5):
                            for gi in main_gis:
                                rhs = (xtds[gi][:, kb, :] if kb < 4
                                       else st["sc4"][gi][:, :])
                                nc.tensor.matmul(
                                    p1s[gi][:], w1s[:, kb, hsl], rhs,
                                    start=(kb == 0), stop=(kb == 4))
                        for gi in main_gis:
                            nc.scalar.activation(
                                h1s[gi][:, hb, :], p1s[gi][:], Silu,
                                scale=1.0 / math.sqrt(640.0))
                    for gi in tail_gis:
                        eg = groups[gi][1]
                        for hb in range(4):
                            hsl = slice(hb * 128, (hb + 1) * 128)
                            p1 = pMP.tile([128, 512], f32, tag="mlp")
                            for kb in range(5):
                                rhs = (xtds[gi][:, kb, :eg] if kb < 4
                                       else st["sc4"][gi][:, :eg])
                                nc.tensor.matmul(
                                    p1[:, :eg], w1s[:, kb, hsl], rhs,
                                    start=(kb == 0), stop=(kb == 4))
                            nc.scalar.activation(
                                h1s[gi][:, hb, :eg], p1[:, :eg], Silu,
                                scale=1.0 / math.sqrt(640.0))
                    # W2
                    h2s = [pH2.tile([128, 4, 512], bf16, tag="h2", name=f"h2_{i}")
                           for i in range(NG)]
                    for hb in range(4):
                        hsl = slice(hb * 128, (hb + 1) * 128)
                        p2s = {gi: pMP.tile([128, 512], f32, tag="mlp",
                                            name=f"p2_{gi}")
                               for gi in main_gis}
                        for kb in range(4):
                            for gi in main_gis:
                                nc.tensor.matmul(
                                    p2s[gi][:], w2s[:, kb, hsl],
                                    h1s[gi][:, kb, :],
                                    start=(kb == 0), stop=(kb == 3))
                        for gi in main_gis:
                            nc.scalar.activation(
                                h2s[gi][:, hb, :], p2s[gi][:], Silu,
                                scale=1.0 / math.sqrt(512.0))
                    for gi in tail_gis:
                        eg = groups[gi][1]
                        for hb in range(4):
                            hsl = slice(hb * 128, (hb + 1) * 128)
                            p2 = pMP.tile([128, 512], f32, tag="mlp")
                            for kb in range(4):
                                nc.tensor.matmul(
                                    p2[:, :eg], w2s[:, kb, hsl],
                                    h1s[gi][:, kb, :eg],
                                    start=(kb == 0), stop=(kb == 3))
                            nc.scalar.activation(
                                h2s[gi][:, hb, :eg], p2[:, :eg], Silu,
                                scale=1.0 / math.sqrt(512.0))
                    # W3 + envelope
                    for gi, (e0_, eg) in enumerate(groups):
                        for es in range(eg // 128):
                            gt = (c * SPC + e0_) // 128 + es
                            p3 = pMP.tile([128, 512], f32, tag="mlp")
                            for kb in range(4):
                                nc.tensor.matmul(
                                    p3[:],
                                    h2s[gi][:, kb, es * 128:(es + 1) * 128],
                                    w3s[:, kb],
                                    start=(kb == 0), stop=(kb == 3))
                            xo = pXo.tile([128, 512], bf16)
                            nc.scalar.activation(
                                xo[:], p3[:], Copy,
                                scale=env_all[:, gt:gt + 1])
                            nc.scalar.dma_start(
                                out1[gt * 128:(gt + 1) * 128, :], xo[:])

                prev = None
                for c in range(CHUNKS_PER_CORE):
                    st = emit_front1(c)
                    if prev is not None:
                        emit_vout(prev)
                    emit_front2(st)
                    if prev is not None:
                        emit_back(prev)
                    prev = st
                emit_vout(prev)
                emit_back(prev)
    nc.compile()
    _GRAPH_CACHE[T] = (nc, NT, E_PAD, SPC)
    return _GRAPH_CACHE[T]


def _prep_host(vectors, x, V, senders, W_w, W1, W2, W3, W_lin):
    import ml_dtypes
    bf = ml_dtypes.bfloat16
    E = senders.shape[0]
    gchunk = senders.astype(np.int64) // CHUNK_NODES
    order = np.argsort(gchunk, kind="stable")
    counts = np.bincount(gchunk, minlength=64)
    T = max(17, int(math.ceil(counts.max() / 128.0)))
    SPC = T * 128
    NT = CHUNKS_PER_CORE * T
    E_PAD = NT * 128

    sg = gchunk[order]
    starts = np.zeros(64, np.int64)
    starts[1:] = np.cumsum(counts)[:-1]
    within = np.arange(E, dtype=np.int64) - starts[sg]
    core = sg // CHUNKS_PER_CORE
    lchunk = sg % CHUNKS_PER_CORE
    slot = lchunk * SPC + within

    W1s = W1.astype(np.float64)
    W1s[576:640] /= math.sqrt(3.0)
    W1p = np.ascontiguousarray(
        W1s.reshape(5, 128, 512).transpose(1, 0, 2)).astype(bf)
    W2p = np.ascontiguousarray(
        W2.reshape(4, 128, 512).transpose(1, 0, 2)).astype(bf)
    W3p = np.ascontiguousarray(
        W3.reshape(4, 128, 512).transpose(1, 0, 2)).astype(bf)
    Wwp = np.ascontiguousarray(
        (W_w.astype(np.float64) * (EPS / math.sqrt(512.0)))
        .reshape(4, 128, 64).transpose(1, 0, 2)).astype(bf)
    WLs = W_lin.astype(np.float64) / math.sqrt(192.0)
    WLs[128:192] /= math.sqrt(2.0)
    WLp = np.ascontiguousarray(
        WLs.reshape(3, 64, 64).transpose(1, 0, 2)).astype(bf)

    in_maps = []
    metas = []
    for d in range(N_CORES):
        m = core == d
        eidx = order[m]
        sl = slot[m]
        fill = eidx[0] if len(eidx) else 0
        xs = np.empty((E_PAD, 512), np.float32)
        xs[:] = x[fill]
        xs[sl] = x[eidx]
        vs = np.empty((E_PAD, 3), np.float32)
        vs[:] = vectors[fill]
        vs[sl] = vectors[eidx]
        Vs = np.empty((E_PAD, 64, 4), np.float32)
        Vs[:] = V[fill]
        Vs[sl] = V[eidx]
        xTc = np.ascontiguousarray(xs.T).astype(bf)
        VTmc = np.ascontiguousarray(Vs.transpose(1, 2, 0)).astype(bf)
        Sc = np.zeros((NT, 128, 128), bf)
        col = (senders[eidx] % CHUNK_NODES).astype(np.int64)
        Sc[sl // 128, sl % 128, col] = 1.0
        STgc = np.zeros((CHUNKS_PER_CORE, 128, SPC), bf)
        STgc[sl // SPC, col, sl % SPC] = 1.0
        in_maps.append({
            "xT": xTc, "vec": vs, "VTm": VTmc, "S": Sc, "STg": STgc,
            "W1p": W1p, "W2p": W2p, "W3p": W3p, "Wwp": Wwp, "WLp": WLp,
        })
        metas.append((eidx, sl))
    return T, in_maps, metas


def kernel(vectors, x, V, senders, W_w, W1, W2, W3, W_lin):
    vectors = np.asarray(vectors, np.float32)
    x = np.asarray(x, np.float32)
    V = np.asarray(V, np.float32)
    senders = np.asarray(senders)
    W_w = np.asarray(W_w, np.float32)
    W1 = np.asarray(W1, np.float32)
    W2 = np.asarray(W2, np.float32)
    W3 = np.asarray(W3, np.float32)
    W_lin = np.asarray(W_lin, np.float32)

    T, in_maps, metas = _prep_host(
        vectors, x, V, senders, W_w, W1, W2, W3, W_lin)
    nc, NT, E_PAD, SPC = _build_graph(T)

    from concourse.bass_utils import run_bass_kernel_spmd
    res = run_bass_kernel_spmd(nc, in_maps, core_ids=list(range(N_CORES)))

    E = senders.shape[0]
    x_out = np.empty((E, 512), np.float32)
    V_out = np.empty((E, 64, 3), np.float32)
    for d in range(N_CORES):
        eidx, sl = metas[d]
        o1 = res.results[d]["out1"]
        o2 = res.results[d]["out2"]
        x_out[eidx] = o1[sl].astype(np.float32)
        V_out[eidx] = o2.transpose(2, 0, 1)[sl].astype(np.float32)
    return x_out, V_out
